# revision 1
# baseline (speedup 1.0000x reference)
"""Trainium2 Bass kernel for the distributed CLIP-style contrastive loss.

loss = 0.5 * ( mean_i( LSE_row(i) - diag(i) ) + mean_j( LSE_col(j) - diag(j) ) )
with logits = tau * ftir @ raman.T, tau = min(exp(log_tau), 100), B=4096, D=512.

Sharding: rows of the [B, B] logits matrix are split across 8 cores (512 rows
each).  Each core computes BOTH its row-slab of logits (ftir_shard @ raman.T)
and its row-slab of logits.T (raman_shard @ ftir.T), so the column-softmax is
just a second row-softmax and no collectives are needed.  Row log-sum-exp is
computed with an exact two-level scheme: per 1024-wide block the VectorE takes
the block max straight out of PSUM, the ScalarE computes exp(x - m_b) with a
fused free-dim accumulation (accum_out), and a tiny per-row fixup combines the
block partial sums:  LSE = M + log(sum_b s_b * exp(m_b - M)).

Each core returns per-row LSEs ([128, 8]) and the diagonal dot products
([1, 512]); the host does the final scalar reduction.
"""

import sys

import numpy as np

for _p in ("/opt/trn_rl_repo", "/root/.axon_site/_ro/trn_rl_repo"):
    if _p not in sys.path:
        sys.path.append(_p)

from contextlib import ExitStack

import concourse.bacc as bacc
import concourse.tile as tile
from concourse import mybir
from concourse.bass_utils import run_bass_kernel_spmd

B = 4096
D = 512
NCORES = 8
SH = B // NCORES  # 512 rows per core
P = 128
KC = D // P  # 4 k-chunks of 128
MT = SH // P  # 4 m-tiles of 128 rows
BLK = 1024  # PSUM stats-block width
NB = B // BLK  # 4 blocks per row
SUB = 512  # matmul N per instruction
CHW = 2048  # DMA chunk width for the full tensors
NCH = B // CHW  # 2 chunks per k-slice

# matmul input dtype: bfloat16 (fast, half DMA) or float32r (full-rate fp32
# streaming mode) or float32 (4x slower matmul).
DT_IN = mybir.dt.bfloat16

F32 = mybir.dt.float32
AX = mybir.AxisListType
ALU = mybir.AluOpType
ACTF = mybir.ActivationFunctionType

# toggled by test harness for profiling
PROFILE = False
LAST_RESULTS = None

_prog_cache = {}


def _build_program(dt_in):
    nc = bacc.Bacc("TRN2", target_bir_lowering=False, debug=False)

    ats = nc.dram_tensor("ats", [D, SH], dt_in, kind="ExternalInput").ap()
    bts = nc.dram_tensor("bts", [D, SH], dt_in, kind="ExternalInput").ap()
    atf = nc.dram_tensor("atf", [D, B], dt_in, kind="ExternalInput").ap()
    btf = nc.dram_tensor("btf", [D, B], dt_in, kind="ExternalInput").ap()
    lse_out = nc.dram_tensor("lse", [P, 2 * MT], F32, kind="ExternalOutput").ap()
    diag_out = nc.dram_tensor("diag", [1, SH], F32, kind="ExternalOutput").ap()

    with ExitStack() as ctx:
        tc = ctx.enter_context(tile.TileContext(nc))
        inp = ctx.enter_context(tc.tile_pool(name="inp", bufs=1))
        psum = ctx.enter_context(tc.tile_pool(name="psum", bufs=3, space="PSUM"))
        dpsum = ctx.enter_context(tc.tile_pool(name="dpsum", bufs=1, space="PSUM"))
        scr = ctx.enter_context(tc.tile_pool(name="scr", bufs=3))
        stats = ctx.enter_context(tc.tile_pool(name="stats", bufs=2))
        small = ctx.enter_context(tc.tile_pool(name="small", bufs=2))

        # ---- persistent input tiles ----
        a_sh = inp.tile([P, KC, SH], dt_in, tag="ash")
        b_sh = inp.tile([P, KC, SH], dt_in, tag="bsh")
        nc.sync.dma_start(
            out=a_sh, in_=ats.rearrange("(k p) n -> p k n", p=P)
        )
        nc.sync.dma_start(
            out=b_sh, in_=bts.rearrange("(k p) n -> p k n", p=P)
        )

        # full tensors as separate chunk tiles for fine-grained DMA deps
        def chunked_load(src, name):
            tiles = []
            for k in range(KC):
                row = []
                for ch in range(NCH):
                    t = inp.tile([P, CHW], dt_in, tag=f"{name}_{k}_{ch}")
                    row.append(t)
                tiles.append(row)
            return tiles

        b_f = chunked_load(btf, "bf")
        a_f = chunked_load(atf, "af")
        # issue DMAs in consumption order: all of b (used by pass L1) first
        for ch in range(NCH):
            for k in range(KC):
                nc.sync.dma_start(
                    out=b_f[k][ch],
                    in_=btf[k * P : (k + 1) * P, ch * CHW : (ch + 1) * CHW],
                )
        for ch in range(NCH):
            for k in range(KC):
                nc.sync.dma_start(
                    out=a_f[k][ch],
                    in_=atf[k * P : (k + 1) * P, ch * CHW : (ch + 1) * CHW],
                )

        lse_sb = inp.tile([P, 2 * MT], F32, tag="lse_sb")

        # ---- diagonal: diag[i] = sum_d a_sh[d, i] * b_sh[d, i] ----
        # elementwise mul on VE, then partition-sum via a ones-matmul.
        ones = inp.tile([P, 1], dt_in, tag="ones")
        nc.vector.memset(ones, 1.0)
        dps = dpsum.tile([1, SH], F32)
        for k in range(KC):
            prod = scr.tile([P, SH], dt_in, tag="prod")
            nc.vector.tensor_mul(prod, a_sh[:, k, :], b_sh[:, k, :])
            nc.tensor.matmul(
                dps, lhsT=ones, rhs=prod, start=(k == 0), stop=(k == KC - 1)
            )
        diag_sb = small.tile([1, SH], F32, tag="diag_sb")
        nc.scalar.copy(diag_sb, dps)
        nc.sync.dma_start(out=diag_out, in_=diag_sb)

        # ---- main two passes ----
        for L in range(2):
            lhs = a_sh if L == 0 else b_sh
            rhs_t = b_f if L == 0 else a_f
            for m in range(MT):
                negm = stats.tile([P, NB], F32, tag="negm")
                sums = stats.tile([P, NB], F32, tag="sums")
                for t in range(NB):
                    ps = psum.tile([P, BLK], F32, tag="ps")
                    for j in range(BLK // SUB):
                        n0 = t * BLK + j * SUB
                        chi, off = divmod(n0, CHW)
                        for k in range(KC):
                            nc.tensor.matmul(
                                ps[:, j * SUB : (j + 1) * SUB],
                                lhsT=lhs[:, k, m * P : (m + 1) * P],
                                rhs=rhs_t[k][chi][:, off : off + SUB],
                                start=(k == 0),
                                stop=(k == KC - 1),
                            )
                    # block stats straight from PSUM
                    nc.vector.reduce_max(
                        out=negm[:, t : t + 1], in_=ps, axis=AX.X, negate=True
                    )
                    sc = scr.tile([P, BLK], F32, tag="escr")
                    nc.scalar.activation(
                        sc,
                        ps,
                        ACTF.Exp,
                        bias=negm[:, t : t + 1],
                        accum_out=sums[:, t : t + 1],
                    )
                # per-row fixup: LSE = M + log(sum_b s_b * exp(m_b - M))
                nmin = small.tile([P, 1], F32, tag="nmin")
                nc.vector.tensor_reduce(out=nmin, in_=negm, axis=AX.X, op=ALU.min)
                f = small.tile([P, NB], F32, tag="f")
                nc.scalar.activation(f, negm, ACTF.Exp, bias=nmin, scale=-1.0)
                w = small.tile([P, NB], F32, tag="w")
                nc.vector.tensor_mul(w, f, sums)
                stot = small.tile([P, 1], F32, tag="stot")
                nc.vector.reduce_sum(out=stot, in_=w, axis=AX.X)
                lg = small.tile([P, 1], F32, tag="lg")
                nc.scalar.activation(lg, stot, ACTF.Ln)
                nc.vector.tensor_sub(
                    lse_sb[:, L * MT + m : L * MT + m + 1], lg, nmin
                )

        nc.sync.dma_start(out=lse_out, in_=lse_sb)

    nc.compile()
    return nc


def _get_program(dt_in):
    key = str(dt_in)
    if key not in _prog_cache:
        _prog_cache[key] = _build_program(dt_in)
    return _prog_cache[key]


def kernel(out_ftir, out_raman, labels=None, log_tau=None, **_unused):
    global LAST_RESULTS
    out_ftir = np.asarray(out_ftir, dtype=np.float32)
    out_raman = np.asarray(out_raman, dtype=np.float32)
    tau = float(np.minimum(np.exp(np.float64(np.asarray(log_tau))), 100.0))

    np_dt = mybir.dt.np(DT_IN)
    aT = np.ascontiguousarray((out_ftir * np.float32(tau)).T).astype(np_dt)
    bT = np.ascontiguousarray(out_raman.T).astype(np_dt)

    in_maps = []
    for c in range(NCORES):
        sl = slice(c * SH, (c + 1) * SH)
        in_maps.append(
            {
                "ats": np.ascontiguousarray(aT[:, sl]),
                "bts": np.ascontiguousarray(bT[:, sl]),
                "atf": aT,
                "btf": bT,
            }
        )

    nc = _get_program(DT_IN)
    res = run_bass_kernel_spmd(
        nc, in_maps, core_ids=list(range(NCORES)), trace=PROFILE
    )
    LAST_RESULTS = res

    s_lse = 0.0
    s_diag = 0.0
    for r in res.results:
        s_lse += float(r["lse"].astype(np.float64).sum())
        s_diag += float(r["diag"].astype(np.float64).sum())
    loss = (s_lse - 2.0 * s_diag) / (2.0 * B)
    return np.array(loss, dtype=np.float32)


# revision 6
# speedup vs baseline: 1.1565x; 1.1565x over previous
"""Trainium2 Bass kernel for the distributed CLIP-style contrastive loss.

loss = 0.5 * ( mean_i( LSE_row(i) - diag(i) ) + mean_j( LSE_col(j) - diag(j) ) )
with logits = tau * ftir @ raman.T, tau = min(exp(log_tau), 100), B=4096, D=512.

Sharding: rows of the [B, B] logits matrix are split across 8 cores (512 rows
each).  Each core computes BOTH its row-slab of logits (ftir_shard @ raman.T)
and its row-slab of logits.T (raman_shard @ ftir.T), so the column-softmax is
just a second row-softmax and no collectives are needed.  Row log-sum-exp is
computed with an exact two-level scheme: per 1024-wide block the VectorE takes
the block max straight out of PSUM, the ScalarE computes exp(x - m_b) with a
fused free-dim accumulation (accum_out), and a tiny per-row fixup combines the
block partial sums:  LSE = M + log(sum_b s_b * exp(m_b - M)).

Each core returns per-row LSEs ([128, 8]) and the diagonal dot products
([1, 512]); the host does the final scalar reduction.
"""

import sys

import numpy as np

for _p in ("/opt/trn_rl_repo", "/root/.axon_site/_ro/trn_rl_repo"):
    if _p not in sys.path:
        sys.path.append(_p)

from contextlib import ExitStack

import concourse.bacc as bacc
import concourse.tile as tile
from concourse import mybir
from concourse.bass_utils import run_bass_kernel_spmd

B = 4096
D = 512
NCORES = 8
SH = B // NCORES  # 512 rows per core
P = 128
KC = D // P  # 4 k-chunks of 128
MT = SH // P  # 4 m-tiles of 128 rows
BLK = 1024  # PSUM stats-block width
NB = B // BLK  # 4 blocks per row
SUB = 512  # matmul N per instruction
CHW = 2048  # DMA chunk width for the full tensors
NCH = B // CHW  # 2 chunks per k-slice

# matmul input dtype: bfloat16 (fast, half DMA) or float32r (full-rate fp32
# streaming mode) or float32 (4x slower matmul).
DT_IN = mybir.dt.bfloat16

F32 = mybir.dt.float32
AX = mybir.AxisListType
ALU = mybir.AluOpType
ACTF = mybir.ActivationFunctionType

# toggled by test harness for profiling
PROFILE = False
LAST_RESULTS = None

_prog_cache = {}


def _build_program(dt_in):
    nc = bacc.Bacc(
        "TRN2", target_bir_lowering=False, debug=False, enable_partition_id=False
    )

    ats = nc.dram_tensor("ats", [D, SH], dt_in, kind="ExternalInput").ap()
    bts = nc.dram_tensor("bts", [D, SH], dt_in, kind="ExternalInput").ap()
    atf = nc.dram_tensor("atf", [D, B], dt_in, kind="ExternalInput").ap()
    btf = nc.dram_tensor("btf", [D, B], dt_in, kind="ExternalInput").ap()
    lse_out = nc.dram_tensor("lse", [P, 2 * MT], F32, kind="ExternalOutput").ap()
    diag_out = nc.dram_tensor("diag", [1, SH], F32, kind="ExternalOutput").ap()

    with ExitStack() as ctx:
        tc = ctx.enter_context(tile.TileContext(nc))
        inp = ctx.enter_context(tc.tile_pool(name="inp", bufs=1))
        psum = ctx.enter_context(tc.tile_pool(name="psum", bufs=3, space="PSUM"))
        dpsum = ctx.enter_context(tc.tile_pool(name="dpsum", bufs=1, space="PSUM"))
        scr = ctx.enter_context(tc.tile_pool(name="scr", bufs=3))
        stats = ctx.enter_context(tc.tile_pool(name="stats", bufs=2))
        small = ctx.enter_context(tc.tile_pool(name="small", bufs=2))

        # ---- PE warm-up: dummy matmuls while input DMAs stream in. ----
        # Keeps TensorE busy through the DMA-bound head so HAM reaches
        # K=8/8 before the first real matmul (else ~25 MMs run at 1.2GHz).
        warm_sb = inp.tile([P, SUB], dt_in, tag="warm_sb")
        nc.vector.memset(warm_sb, 0.0)
        warm_ps = dpsum.tile([P, SUB], F32, tag="warm_ps")
        for _ in range(14):
            nc.tensor.matmul(
                warm_ps, lhsT=warm_sb[:, :P], rhs=warm_sb, start=True, stop=True
            )

        # ---- persistent input tiles ----
        a_sh = inp.tile([P, KC, SH], dt_in, tag="ash")
        b_sh = inp.tile([P, KC, SH], dt_in, tag="bsh")

        # full tensors as separate chunk tiles for fine-grained DMA deps
        def chunked_alloc(name):
            tiles = []
            for k in range(KC):
                row = []
                for ch in range(NCH):
                    t = inp.tile([P, CHW], dt_in, tag=f"{name}_{k}_{ch}")
                    row.append(t)
                tiles.append(row)
            return tiles

        b_f = chunked_alloc("bf")
        a_f = chunked_alloc("af")

        # issue DMAs in consumption order: first-block deps (a_sh + b ch0)
        # first, b_sh (needed by diag/L2) after them.
        nc.sync.dma_start(out=a_sh, in_=ats.rearrange("(k p) n -> p k n", p=P))
        for k in range(KC):
            nc.sync.dma_start(
                out=b_f[k][0], in_=btf[k * P : (k + 1) * P, 0:CHW]
            )
        nc.sync.dma_start(out=b_sh, in_=bts.rearrange("(k p) n -> p k n", p=P))
        for ch in range(1, NCH):
            for k in range(KC):
                nc.sync.dma_start(
                    out=b_f[k][ch],
                    in_=btf[k * P : (k + 1) * P, ch * CHW : (ch + 1) * CHW],
                )
        for ch in range(NCH):
            for k in range(KC):
                nc.sync.dma_start(
                    out=a_f[k][ch],
                    in_=atf[k * P : (k + 1) * P, ch * CHW : (ch + 1) * CHW],
                )

        lse_sb = inp.tile([P, 2 * MT], F32, tag="lse_sb")
        # per-(L,m) staging so the Ln runs ONCE at the end (a mid-stream Ln
        # forces an Exp<->Ln ACT-table reload per m-row, 2x1.28us each, which
        # stalls the PSUM drain chain and with it the PE).
        nmin_all = inp.tile([P, 2 * MT], F32, tag="nmin_all")
        stot_all = inp.tile([P, 2 * MT], F32, tag="stot_all")

        # ---- diagonal: diag[i] = sum_d a_sh[d, i] * b_sh[d, i] ----
        # elementwise mul on VE, then partition-sum via a ones-matmul.
        ones = inp.tile([P, 1], dt_in, tag="ones")
        nc.vector.memset(ones, 1.0)
        dps = dpsum.tile([1, SH], F32)
        for k in range(KC):
            prod = scr.tile([P, SH], dt_in, tag="prod")
            nc.vector.tensor_mul(prod, a_sh[:, k, :], b_sh[:, k, :])
            nc.tensor.matmul(
                dps, lhsT=ones, rhs=prod, start=(k == 0), stop=(k == KC - 1)
            )
        diag_sb = small.tile([1, SH], F32, tag="diag_sb")
        nc.scalar.copy(diag_sb, dps)
        nc.sync.dma_start(out=diag_out, in_=diag_sb)

        # ---- main two passes ----
        for L in range(2):
            lhs = a_sh if L == 0 else b_sh
            rhs_t = b_f if L == 0 else a_f
            for m in range(MT):
                negm = stats.tile([P, NB], F32, tag="negm")
                sums = stats.tile([P, NB], F32, tag="sums")
                for t in range(NB):
                    ps = psum.tile([P, BLK], F32, tag="ps")
                    for j in range(BLK // SUB):
                        n0 = t * BLK + j * SUB
                        chi, off = divmod(n0, CHW)
                        for k in range(KC):
                            nc.tensor.matmul(
                                ps[:, j * SUB : (j + 1) * SUB],
                                lhsT=lhs[:, k, m * P : (m + 1) * P],
                                rhs=rhs_t[k][chi][:, off : off + SUB],
                                start=(k == 0),
                                stop=(k == KC - 1),
                            )
                    # block stats straight from PSUM
                    nc.vector.reduce_max(
                        out=negm[:, t : t + 1], in_=ps, axis=AX.X, negate=True
                    )
                    sc = scr.tile([P, BLK], F32, tag="escr")
                    nc.scalar.activation(
                        sc,
                        ps,
                        ACTF.Exp,
                        bias=negm[:, t : t + 1],
                        accum_out=sums[:, t : t + 1],
                    )
                # per-row fixup: LSE = M + log(sum_b s_b * exp(m_b - M)).
                # Only Exp-table work here; the Ln happens batched at the end.
                col = L * MT + m
                nmin = nmin_all[:, col : col + 1]
                nc.vector.tensor_reduce(out=nmin, in_=negm, axis=AX.X, op=ALU.min)
                f = small.tile([P, NB], F32, tag="f")
                nc.scalar.activation(f, negm, ACTF.Exp, bias=nmin, scale=-1.0)
                w = small.tile([P, NB], F32, tag="w")
                nc.vector.tensor_mul(w, f, sums)
                nc.vector.reduce_sum(
                    out=stot_all[:, col : col + 1], in_=w, axis=AX.X
                )

        # batched final: LSE = log(stot) - nmin  (single Ln table load)
        lg_all = small.tile([P, 2 * MT], F32, tag="lg_all")
        nc.scalar.activation(lg_all, stot_all, ACTF.Ln)
        nc.vector.tensor_sub(lse_sb, lg_all, nmin_all)
        nc.sync.dma_start(out=lse_out, in_=lse_sb)

    nc.compile()
    return nc


def _get_program(dt_in):
    key = str(dt_in)
    if key not in _prog_cache:
        _prog_cache[key] = _build_program(dt_in)
    return _prog_cache[key]


def kernel(out_ftir, out_raman, labels=None, log_tau=None, **_unused):
    global LAST_RESULTS
    out_ftir = np.asarray(out_ftir, dtype=np.float32)
    out_raman = np.asarray(out_raman, dtype=np.float32)
    tau = float(np.minimum(np.exp(np.float64(np.asarray(log_tau))), 100.0))

    np_dt = mybir.dt.np(DT_IN)
    aT = np.ascontiguousarray((out_ftir * np.float32(tau)).T).astype(np_dt)
    bT = np.ascontiguousarray(out_raman.T).astype(np_dt)

    in_maps = []
    for c in range(NCORES):
        sl = slice(c * SH, (c + 1) * SH)
        in_maps.append(
            {
                "ats": np.ascontiguousarray(aT[:, sl]),
                "bts": np.ascontiguousarray(bT[:, sl]),
                "atf": aT,
                "btf": bT,
            }
        )

    nc = _get_program(DT_IN)
    res = run_bass_kernel_spmd(
        nc, in_maps, core_ids=list(range(NCORES)), trace=PROFILE
    )
    LAST_RESULTS = res

    s_lse = 0.0
    s_diag = 0.0
    for r in res.results:
        s_lse += float(r["lse"].astype(np.float64).sum())
        s_diag += float(r["diag"].astype(np.float64).sum())
    loss = (s_lse - 2.0 * s_diag) / (2.0 * B)
    return np.array(loss, dtype=np.float32)
